# revision 1
# baseline (speedup 1.0000x reference)
"""Trainium2 Bass kernel: multi-scale masked average-pool descriptors.

Computes, per batch element b and scribble i:
    d_l[b,i,c] = mean over {pixels where resize(scribble)[b,i,y,x] > 0.5} of feat_l[b,c,y,x]
    out[b,i,c] = (d_0 + d_1 + d_2) / 3

Strategy (v4 -- all-measured design):
  * jax.image.resize(bilinear, antialias=False) at scales 4/8/16 reduces to an
    exact 2x2 average at stride k with offset o (k,o) = (4,1)/(8,3)/(16,7):
    mask == ((a+c)+(b+d)) > 2.0 bit-exactly in fp32 (computed on DVE).
    Scribbles ride the gpsimd SWDGE queue as merged 4KB row-pair descriptors.
  * Feature maps are DMA'd with FULL-ROW descriptors ([y, c-group, x] tiles,
    one 512/256/128B descriptor per (c,y) row) on the two HWDGE rings -- the
    DMA descriptor walk performs the [c,y,x] -> [y,...] partition transpose
    for free (~233 GB/s measured; the xbar and PE transpose alternatives
    measured slower and/or serialize against all other DMA).
  * Engine copies re-pack each c-group [y, 32c, w] fp32 into assembled
    [y, x, 256c] bf16 tiles (cast during copy), so every matmul rhs is a
    contiguous [h, 256] bf16 slice.
  * ssum[i,:] accumulates as one matmul per pixel column x: lhsT =
    mask[:, :, x] [h, 16] bf16, rhs = f[:, x, :] [h, 256] bf16 -- measured
    251ns per LDWEIGHTS+MATMUL pair (strided rhs would be 779ns).
  * cnt[i] = reduce_sum over the mask + a ones-matmul; bf16 masks are exact
    0/1 and PSUM accumulates fp32, so cnt is exact and masks match the
    reference bit-exactly.  bf16 features give rel err ~2e-3 (gate: 2e-2).
  * The empty-mask fallback is handled on the host (P(empty) ~ 2^-1024).

Sharding: pure data-parallel over batch B=8 across the 8 NeuronCores.
"""

import numpy as np

_B = 8
_I = 16
_C = 256

# level: (h, k, off)
_LEVELS = {0: (128, 4, 1), 1: (64, 8, 3), 2: (32, 16, 7)}


def _build_nc():
    import concourse.bacc as bacc
    import concourse.tile as tile
    from concourse import mybir

    f32 = mybir.dt.float32
    bf16 = mybir.dt.bfloat16
    gt = mybir.AluOpType.is_gt
    X = mybir.AxisListType.X

    nc = bacc.Bacc("TRN2", target_bir_lowering=False, debug=False)

    feats = {
        0: nc.dram_tensor("feat0", [_C, 128, 128], f32, kind="ExternalInput"),
        1: nc.dram_tensor("feat1", [_C, 64, 64], f32, kind="ExternalInput"),
        2: nc.dram_tensor("feat2", [_C, 32, 32], f32, kind="ExternalInput"),
    }
    scr = nc.dram_tensor("scribbles", [_I, 512, 512], f32, kind="ExternalInput")
    out_d = nc.dram_tensor("out", [_I, 3 * (_C + 1)], f32, kind="ExternalOutput")

    with tile.TileContext(nc) as tc:
        with (
            tc.tile_pool(name="singles", bufs=1) as singles,
            tc.tile_pool(name="scrib", bufs=2) as scrib,
            tc.tile_pool(name="scrib2", bufs=3) as scrib2,
            tc.tile_pool(name="tmp", bufs=2) as tmp,
            tc.tile_pool(name="fR", bufs=4) as fR,
            tc.tile_pool(name="psum", bufs=3, space="PSUM") as psum,
        ):
            ones = singles.tile([128, 1], f32, tag="ones")
            nc.vector.memset(ones[:], 1.0)
            stag = singles.tile([_I, 3 * (_C + 1)], f32, tag="stag")

            # masks, y-on-partitions (natural resize layout): msk_l[y, i, x]
            msk0 = singles.tile([128, _I, 128], bf16, tag="msk0")
            msk1 = singles.tile([64, _I, 64], bf16, tag="msk1")
            msk2 = singles.tile([32, _I, 32], bf16, tag="msk2")
            msk = {0: msk0, 1: msk1, 2: msk2}
            # assembled feature tiles [y, x, c] bf16
            sgT0 = singles.tile([128, 128, _C], bf16, tag="sgT0")
            sgT1 = singles.tile([64, 64, _C], bf16, tag="sgT1")
            sgT2 = singles.tile([32, 32, _C], bf16, tag="sgT2")
            sgT = {0: sgT0, 1: sgT1, 2: sgT2}

            # ---- interleaved per-level streams ----------------------
            # Queues: gpsimd = scribbles (4KB row-pair descs), sync/scalar =
            # feature full-row loads.  The DVE FIFO alternates one feature
            # assembly copy with one tile's mask ALU so neither stream
            # stalls the other; emission order == engine FIFO order.

            def mask_ops(li, i, st, il=None):
                # only the 2-of-k needed resize columns are added (strided)
                h, k, off = _LEVELS[li]
                src_lo = st[:, il, 0:512] if il is not None else st[:, 0, :]
                src_hi = st[:, il, 512:1024] if il is not None else st[:, 1, :]
                a = src_lo.rearrange("p (x k) -> p x k", k=k)[:, :, off : off + 2]
                b = src_hi.rearrange("p (x k) -> p x k", k=k)[:, :, off : off + 2]
                v = tmp.tile([h, h, 2], f32, tag="v")
                nc.vector.tensor_add(v[:], a, b)
                sr = tmp.tile([h, h], f32, tag="sr")
                nc.vector.tensor_add(sr[:], v[:, :, 0], v[:, :, 1])
                nc.vector.tensor_scalar(
                    out=msk[li][:, i, :], in0=sr[:], scalar1=2.0,
                    scalar2=None, op0=gt,
                )

            def feat_load_copy(li, g):
                # 16-channel staging groups; loads alternate the two HWDGE
                # rings (descriptor generation is ~1.4ns/desc and must be
                # split).  Copies are cross-assigned -- scalar copies what
                # sync loaded and the DVE copies what scalar loaded -- so a
                # copy never waits on its own engine's queue head.
                h = _LEVELS[li][0]
                sg = fR.tile([h, 16, h], f32, tag="sgR")
                deng = nc.sync if g % 2 == 0 else nc.scalar
                deng.dma_start(
                    out=sg[:],
                    in_=feats[li][16 * g : 16 * (g + 1)].rearrange(
                        "c y x -> y c x"
                    ),
                )
                dst = sgT[li][:, :, 16 * g : 16 * (g + 1)]
                srcv = sg[:].rearrange("p c x -> p x c")
                if g % 2 == 0:
                    nc.scalar.copy(dst, srcv)
                else:
                    nc.vector.tensor_copy(dst, srcv)

            def scrib_load(li, i):
                h, k, off = _LEVELS[li]
                rr = 512 // h
                st = scrib2.tile([h, 2, 512], f32, tag=f"st{li}")
                nc.gpsimd.dma_start(
                    out=st[:],
                    in_=scr[i].rearrange("(y r) x -> y r x", r=rr)[
                        :, off : off + 2, :
                    ],
                )
                return st

            # main loop: L0 features+masks with the L1/L2 scribble+mask
            # streams interleaved round-robin so they finish inside L0's
            # DMA phase instead of trailing it
            for t in range(8):
                i0 = t * 2
                st = scrib.tile([128, 2, 1024], f32, tag="st0")
                nc.gpsimd.dma_start(
                    out=st[:],
                    in_=scr[i0 : i0 + 2]
                    .rearrange("i (y k) x -> y i k x", k=4)[:, :, 1:3, :]
                    .rearrange("y i k x -> y i (k x)"),
                )
                for il in range(2):
                    i = i0 + il
                    feat_load_copy(0, 2 * t + il)
                    mask_ops(0, i, st, il=il)
                    mask_ops(1, i, scrib_load(1, i))
                    mask_ops(2, i, scrib_load(2, i))

            for li in (1, 2):
                for i in range(_I):
                    feat_load_copy(li, i)

            # ---- matmuls + cnt + staging, level order 0, 1, 2
            for li in (0, 1, 2):
                h = _LEVELS[li][0]
                acc = psum.tile([_I, _C], f32, tag="acc")
                for x in range(h):
                    nc.tensor.matmul(
                        acc[:], msk[li][:, :, x], sgT[li][:, x, :],
                        start=(x == 0), stop=(x == h - 1),
                    )
                r = singles.tile([h, _I], f32, tag=f"r{li}")
                nc.vector.reduce_sum(out=r[:], in_=msk[li][:], axis=X)
                cnt = psum.tile([_I, 1], f32, tag="cnt")
                nc.tensor.matmul(cnt[:], r[:], ones[:h, :], start=True, stop=True)
                base = li * (_C + 1)
                nc.vector.tensor_copy(stag[:, base : base + _C], acc[:])
                nc.vector.tensor_copy(stag[:, base + _C : base + _C + 1], cnt[:])

            nc.sync.dma_start(out=out_d[:], in_=stag[:])

    nc.compile()
    return nc


def _host_fallback(scr_bi, fmap_b, h, k, off):
    """Feature at argmax of the soft mask; only used when a mask is empty."""
    V = scr_bi[off::k, :][:h].astype(np.float32) + scr_bi[off + 1 :: k, :][:h]
    sr4 = V[:, off::k][:, :h] + V[:, off + 1 :: k][:, :h]
    idx = int(np.argmax(np.float32(0.25) * sr4))
    y, x = divmod(idx, h)
    return fmap_b[:, y, x]


def kernel(feat0, feat1, feat2, scribbles):
    import sys

    for p in ("/opt/trn_rl_repo", "/opt/pypackages"):
        if p not in sys.path:
            sys.path.append(p)
    from concourse.bass_utils import run_bass_kernel_spmd

    feat0 = np.asarray(feat0, dtype=np.float32)
    feat1 = np.asarray(feat1, dtype=np.float32)
    feat2 = np.asarray(feat2, dtype=np.float32)
    scribbles = np.asarray(scribbles, dtype=np.float32)

    nc = _build_nc()
    in_maps = [
        {
            "feat0": np.ascontiguousarray(feat0[b]),
            "feat1": np.ascontiguousarray(feat1[b]),
            "feat2": np.ascontiguousarray(feat2[b]),
            "scribbles": np.ascontiguousarray(scribbles[b]),
        }
        for b in range(_B)
    ]
    res = run_bass_kernel_spmd(nc, in_maps, core_ids=list(range(_B)))
    raw = np.stack([res.results[b]["out"] for b in range(_B)])  # [B, I, 3*257]
    raw = raw.reshape(_B, _I, 3, _C + 1)
    ssum = raw[..., :_C].astype(np.float32)  # [B, I, 3, C]
    cnt = raw[..., _C].astype(np.float32)  # [B, I, 3]

    mean = ssum / np.maximum(cnt, np.float32(1.0))[..., None]

    if (cnt == 0).any():  # never for non-degenerate inputs
        fm = [feat0, feat1, feat2]
        for b, i, li in zip(*np.nonzero(cnt == 0)):
            h, k, off = _LEVELS[li]
            mean[b, i, li] = _host_fallback(scribbles[b, i], fm[li][b], h, k, off)

    out = (mean[:, :, 0] + mean[:, :, 1] + mean[:, :, 2]) / np.float32(3.0)
    return out.astype(np.float32)

